# revision 21
# baseline (speedup 1.0000x reference)
"""Causal self-attention (B=4, T=2048, C=768, H=12) on 8 trn2 NeuronCores.

Sharding: 4 batches x 2 head-groups = 8 cores. Each core computes the qkv
projection + attention for its 6 heads of one batch element in transposed
layout (q^T,k^T as [hd,T], v as [T,hd] -- zero on-device transposes), a
partial output projection over its 384 y-channels for all T, then pairwise
ReduceScatters (3 splits) sum the two partial projections and hand each core
its rows. All matmul operands are fp16 (PE runs 1 cycle/row, same as f32r,
but 16-bit halves SBUF/DMA traffic and doubles DVE rate); PSUM accumulation
is fp32.

The qkv projection, attention, and output projection are interleaved per
query block so the PE/ACT pipelines fill from ~10us on, and the gpsimd queue
carries only the collectives + final output DMAs so the ReduceScatters
overlap compute.

Host work is limited to slicing/transposing/casting inputs and restacking
outputs.
"""
import numpy as np
from contextlib import ExitStack

import concourse.bass as bass
import concourse.bacc as bacc
import concourse.mybir as mybir
import concourse.tile as tile
from concourse.bass_utils import run_bass_kernel_spmd

B, C, H, HD = 4, 768, 12, 64
N_CORES = 8
LH = H // 2          # local heads per core
F32 = mybir.dt.float32
F16 = mybir.dt.float16
Exp = mybir.ActivationFunctionType.Exp
PAIRS = [[0, 1], [2, 3], [4, 5], [6, 7]]
SPLITS = [12, 4]     # m-chunks (128 rows each) per ReduceScatter


def build_program(T=2048, with_bias_qkv=False, with_bias_proj=False):
    CK = C // 128            # 6 contract chunks of the hidden dim
    QB = min(512, T)         # query block (free dim of S^T)
    NQB = T // QB
    DPB = QB // 128          # 128-wide diagonal strips per query block
    QKCOLS = 2 * LH * HD     # 768 local q+k columns
    VCOLS = LH * HD          # 384 local v columns
    WACOLS = QKCOLS + VCOLS  # 1152
    if T == 2048:
        splits = SPLITS
    else:
        splits = [T // 128]
    assert sum(splits) * 128 == T

    nc = bacc.Bacc("TRN2", target_bir_lowering=False, debug=False,
                   num_devices=N_CORES)
    xT_d = nc.dram_tensor("xT", [C, T], F16, kind="ExternalInput")
    wa_d = nc.dram_tensor("wa", [C, WACOLS], F16, kind="ExternalInput")
    wp_d = nc.dram_tensor("wp", [VCOLS, C], F16, kind="ExternalInput")
    tri_d = nc.dram_tensor("tri", [128, 128], F16, kind="ExternalInput")
    if with_bias_qkv:
        bq_d = nc.dram_tensor("bqkv", [1, WACOLS], F16, kind="ExternalInput")
    if with_bias_proj:
        bp_d = nc.dram_tensor("bp", [1, C], F16, kind="ExternalInput")
    out_d = nc.dram_tensor("out", [T // 2, C], F32, kind="ExternalOutput")

    with tile.TileContext(nc) as tc, ExitStack() as top:
        persist = top.enter_context(tc.tile_pool(name="persist", bufs=1))
        dram = top.enter_context(tc.tile_pool(name="dram", bufs=1, space="DRAM"))

        # persistent tensors
        qkT = [persist.tile([128, T], F16, tag=f"qkT{j}", name=f"qkT{j}") for j in range(CK)]
        kTs = [persist.tile([128, T], F16, tag=f"kTs{j}", name=f"kTs{j}") for j in range(3)]
        yT = [persist.tile([128, T], F16, tag=f"yT{j}", name=f"yT{j}") for j in range(3)]
        v_sb = [persist.tile([128, LH * (HD + 1)], F16, tag=f"v{m}", name=f"v{m}")
                for m in range(T // 128)]
        xt = [persist.tile([128, T], F16, tag=f"xt{i}", name=f"xt{i}") for i in range(CK)]
        wa = [persist.tile([128, WACOLS], F16, tag=f"wa{i}", name=f"wa{i}") for i in range(CK)]
        wp = [persist.tile([128, C], F16, tag=f"wp{j}", name=f"wp{j}") for j in range(3)]
        tri = persist.tile([128, 128], F16, tag="tri")
        nc.sync.dma_start(tri[:], tri_d.ap())
        onescol = persist.tile([128, LH], F16, tag="onescol")
        onescol_f = persist.tile([128, LH], F32, tag="onescol_f")
        nc.vector.memset(onescol_f[:], 1.0)
        nc.vector.tensor_copy(onescol[:], onescol_f[:])
        if with_bias_qkv:
            bq_sb = persist.tile([1, WACOLS], F16, tag="bq")
            nc.sync.dma_start(bq_sb[:], bq_d.ap())
            onesq = persist.tile([1, QB], F16, tag="onesq")
            onesq_f = persist.tile([1, QB], F32, tag="onesq_f")
            nc.vector.memset(onesq_f[:], 1.0)
            nc.vector.tensor_copy(onesq[:], onesq_f[:])
        if with_bias_proj:
            bp_sb = persist.tile([1, C], F16, tag="bp")
            nc.sync.dma_start(bp_sb[:], bp_d.ap())
        if with_bias_qkv or with_bias_proj:
            ones128 = persist.tile([1, 128], F16, tag="ones128")
            ones128_f = persist.tile([1, 128], F32, tag="ones128_f")
            nc.vector.memset(ones128_f[:], 1.0)
            nc.vector.tensor_copy(ones128[:], ones128_f[:])

        # input DMAs, sliced per query block so block 0's chains start early
        for i in range(CK):
            nc.sync.dma_start(wa[i][:], wa_d.ap()[128 * i:128 * (i + 1), :])
        for n in range(NQB):
            for i in range(CK):
                nc.sync.dma_start(xt[i][:, QB * n:QB * (n + 1)],
                                  xT_d.ap()[128 * i:128 * (i + 1),
                                            QB * n:QB * (n + 1)])
        for j in range(3):
            nc.sync.dma_start(wp[j][:], wp_d.ap()[128 * j:128 * (j + 1), :])

        # collectives state
        partials = [dram.tile([128 * c, C], F16, name=f"partial{i}")
                    for i, c in enumerate(splits)]
        rs_outs = [dram.tile([64 * c, C], F16, name=f"rs{i}")
                   for i, c in enumerate(splits)]
        split_of_m = []
        for i, c in enumerate(splits):
            split_of_m += [i] * c
        split_base = [0] * len(splits)
        for i in range(1, len(splits)):
            split_base[i] = split_base[i - 1] + splits[i - 1]
        rs_done = [0] * len(splits)
        out_row = [0] * (len(splits) + 1)
        for i, c in enumerate(splits):
            out_row[i + 1] = out_row[i] + 64 * c

        with tc.tile_pool(name="phB", bufs=6) as phB, \
             tc.tile_pool(name="phBs", bufs=4) as phBs, \
             tc.tile_pool(name="stg", bufs=4) as stg, \
             tc.tile_pool(name="psS", bufs=3, space="PSUM") as psS, \
             tc.tile_pool(name="psY", bufs=2, space="PSUM") as psY:

            # ACT spline-table preload: a throwaway exp overlapping the input
            # DMAs instead of stalling the first real softmax tile
            dummy = phBs.tile([1, 16], F32, tag="dummy", bufs=1)
            nc.vector.memset(dummy[:], 0.0)
            dume = phBs.tile([1, 16], F32, tag="dume", bufs=1)
            nc.scalar.activation(dume[:], dummy[:], Exp, scale=0.125)

            # warm the PE clock-gate while input DMAs stream
            for w in range(8):
                wps = psS.tile([128, 2 * QB], F32, tag="sps", bufs=3,
                               name=f"warm{w}")
                nc.tensor.matmul(wps[:, 0:128], tri[:], tri[:],
                                 start=True, stop=True)

            def v_chain(m):
                vps = psY.tile([128, QB], F32, tag="yps", name=f"vps{m}")
                for i in range(CK):
                    nc.tensor.matmul(
                        vps[:, 0:VCOLS], xt[i][:, 128 * m:128 * (m + 1)],
                        wa[i][:, QKCOLS:WACOLS],
                        start=(i == 0),
                        stop=(i == CK - 1 and not with_bias_qkv))
                if with_bias_qkv:
                    nc.tensor.matmul(vps[:, 0:VCOLS], ones128[:],
                                     bq_sb[:, QKCOLS:WACOLS],
                                     start=False, stop=True)
                nc.vector.tensor_copy(
                    v_sb[m][:].rearrange("p (h c) -> p h c", c=HD + 1)[:, :, 0:HD],
                    vps[:, 0:VCOLS].rearrange("p (h c) -> p h c", c=HD))
                nc.vector.tensor_copy(
                    v_sb[m][:].rearrange("p (h c) -> p h c", c=HD + 1)[:, :, HD:HD + 1],
                    onescol[:].rearrange("p (h c) -> p h c", c=1))

            for n in range(NQB):
                # ---- phase A for this block: q_j + k_j chains together so
                # head pair (2j, 2j+1) unblocks as soon as its pair is done
                for jp in range(3):
                    qpw = psS.tile([128, 2 * QB], F32, tag="sps", bufs=3,
                                   name=f"qp{n}_{jp}")
                    for half, j in ((0, jp), (1, 3 + jp)):
                        for i in range(CK):
                            nc.tensor.matmul(
                                qpw[:, QB * half:QB * (half + 1)],
                                wa[i][:, 128 * j:128 * (j + 1)],
                                xt[i][:, QB * n:QB * (n + 1)],
                                start=(i == 0),
                                stop=(i == CK - 1 and not with_bias_qkv))
                        if with_bias_qkv:
                            nc.tensor.matmul(
                                qpw[:, QB * half:QB * (half + 1)],
                                bq_sb[:, 128 * j:128 * (j + 1)], onesq[:],
                                start=False, stop=True)
                        nc.vector.tensor_copy(
                            qkT[j][:, QB * n:QB * (n + 1)],
                            qpw[:, QB * half:QB * (half + 1)])
                    # half-swapped copy of k^T so both PE row groups can host
                    # any head
                    cs = QB * n
                    nc.sync.dma_start(kTs[jp][64:128, cs:cs + QB],
                                      qkT[3 + jp][0:64, cs:cs + QB])
                    nc.sync.dma_start(kTs[jp][0:64, cs:cs + QB],
                                      qkT[3 + jp][64:128, cs:cs + QB])
                for m in range(DPB * n, DPB * (n + 1)):
                    v_chain(m)

                # ---- attention for this block
                nkc = DPB * (n + 1)
                for h in range(LH):
                    jq, rq = h // 2, 64 * (h % 2)
                    klo = qkT[3 + jq] if h % 2 == 0 else kTs[jq]
                    khi = kTs[jq] if h % 2 == 0 else qkT[3 + jq]
                    yps = psY.tile([128, QB], F32, tag="yps",
                                   name=f"yps{n}_{h}")
                    # stage this (head, block)'s q at the opposite base
                    ob = 64 - rq
                    qst = phB.tile([128, QB], F16, tag="qst", bufs=2)
                    nc.sync.dma_start(
                        qst[ob:ob + 64, :],
                        qkT[jq][rq:rq + 64, QB * n:QB * (n + 1)])
                    for kc0 in range(0, nkc, 2):
                        pair = [kc0] if kc0 + 1 >= nkc else [kc0, kc0 + 1]
                        # both S^T tiles of the pair land in one 2-bank psum
                        # tile; adjacent matmuls in distinct PE row groups run
                        # concurrently (K=64 row tiling)
                        spw = psS.tile([128, 2 * QB], F32, tag="sps", bufs=3,
                                       name=f"sp{n}_{h}_{kc0}")
                        ptw = phB.tile([128, 2 * QB], F16, tag="pt", bufs=4)
                        offs = []
                        for pi, kc in enumerate(pair):
                            d = kc - DPB * n
                            c0 = 128 * d if d > 0 else 0
                            off = pi * QB
                            offs.append((kc, d, c0, off))
                            kt, rb = (klo, 0) if kc % 2 == 0 else (khi, 64)
                            if rb == rq:
                                qt_ap = qkT[jq][rq:rq + 64,
                                                QB * n + c0:QB * (n + 1)]
                            else:
                                qt_ap = qst[ob:ob + 64, c0:QB]
                            nc.tensor.matmul(
                                spw[:, off + c0:off + QB],
                                kt[rb:rb + 64, 128 * kc:128 * (kc + 1)],
                                qt_ap, start=True, stop=True)
                        # one wide exp when the garbage prefix is small
                        if len(pair) == 2 and sum(c for _, _, c, _ in offs) <= 256:
                            lo = offs[0][2]
                            nc.scalar.activation(ptw[:, lo:], spw[:, lo:],
                                                 Exp, scale=0.125)
                        else:
                            for kc, d, c0, off in offs:
                                nc.scalar.activation(
                                    ptw[:, off + c0:off + QB],
                                    spw[:, off + c0:off + QB],
                                    Exp, scale=0.125)
                        for kc, d, c0, off in offs:
                            if d >= 0:
                                nc.vector.tensor_tensor(
                                    ptw[:, off + c0:off + c0 + 128],
                                    ptw[:, off + c0:off + c0 + 128],
                                    tri[:], mybir.AluOpType.mult)
                        for kc, d, c0, off in offs:
                            nc.tensor.matmul(
                                yps[0:HD + 1, c0:QB],
                                v_sb[kc][:, (HD + 1) * h:(HD + 1) * (h + 1)],
                                ptw[:, off + c0:off + QB],
                                start=(kc == 0), stop=(kc == nkc - 1))
                    # normalize: yT[.] = yps[0:64] / yps[64].  One staging
                    # copy frees the PSUM tile; reciprocal on DVE; partition
                    # broadcast via a DRAM round-trip DMA (engines cannot
                    # read stride-0 partitions, DMA from DRAM can; gpsimd
                    # stays collective-only)
                    ystg = phBs.tile([HD + 1, QB], F32, tag="ystg", bufs=4,
                                     name=f"ystg{n}_{h}")
                    nc.vector.tensor_copy(ystg[:], yps[0:HD + 1, :])
                    # the custom-DVE reciprocal needs a partition-0 input AP
                    s_sb = phBs.tile([1, QB], F32, tag="s_sb", bufs=4,
                                     name=f"s_sb{n}_{h}")
                    nc.vector.tensor_copy(s_sb[:], yps[HD:HD + 1, :])
                    rec = phBs.tile([1, QB], F32, tag="rec", bufs=4,
                                    name=f"rec{n}_{h}")
                    nc.vector.reciprocal_approx_fast(rec[:], s_sb[:])
                    recd = dram.tile([1, QB], F32, tag="recd", bufs=4,
                                     name=f"recd{n}_{h}")
                    nc.sync.dma_start(recd[:], rec[:])
                    recb = phBs.tile([HD, QB], F32, tag="recb", bufs=4,
                                     name=f"recb{n}_{h}")
                    nc.sync.dma_start(recb[:], recd[:].to_broadcast((HD, QB)))
                    nc.vector.tensor_tensor(
                        yT[jq][rq:rq + 64, QB * n:QB * (n + 1)],
                        ystg[0:HD, :], recb[:], mybir.AluOpType.mult)

                # ---- output projection for this block's row chunks
                for m in range(DPB * n, DPB * (n + 1)):
                    ost = stg.tile([128, C], F16, tag="ost", bufs=4,
                                   name=f"ost{m}")
                    for c0, c1 in ((0, 512), (512, C)):
                        pps = psY.tile([128, QB], F32, tag="yps",
                                       name=f"pp{m}_{c0}")
                        for j in range(3):
                            nc.tensor.matmul(
                                pps[:, 0:c1 - c0],
                                yT[j][:, 128 * m:128 * (m + 1)],
                                wp[j][:, c0:c1], start=(j == 0),
                                stop=(j == 2 and not with_bias_proj))
                        if with_bias_proj:
                            nc.tensor.matmul(pps[:, 0:c1 - c0], ones128[:],
                                             bp_sb[:, c0:c1],
                                             start=False, stop=True)
                        nc.vector.tensor_copy(ost[:, c0:c1],
                                              pps[:, 0:c1 - c0])
                    sp = split_of_m[m]
                    mh = m - split_base[sp]
                    nc.sync.dma_start(
                        partials[sp][128 * mh:128 * (mh + 1), :], ost[:])
                    rs_done[sp] += 1
                    if rs_done[sp] == splits[sp]:
                        nc.gpsimd.collective_compute(
                            "ReduceScatter", mybir.AluOpType.add,
                            replica_groups=PAIRS,
                            ins=[partials[sp].opt()],
                            outs=[rs_outs[sp].opt()])
                        nc.gpsimd.dma_start(
                            out=out_d.ap()[out_row[sp]:out_row[sp + 1], :],
                            in_=rs_outs[sp][:, :])
    nc.compile()
    return nc


def shard_inputs(x, W_attn, b_attn, W_proj, b_proj):
    """Per-core input maps. Core c = 2*b + g handles batch b, head-group g."""
    T = x.shape[1]
    tri = np.tril(np.ones((128, 128), dtype=np.float32)).T.copy()
    # tri[k_row, q_col] = 1 where k <= q  (lower-tri in (q,k) = upper in (k,q))
    tri = tri.astype(np.float16)
    with_bias_qkv = bool(np.any(b_attn))
    with_bias_proj = bool(np.any(b_proj))
    in_maps = []
    for c in range(N_CORES):
        b, g = divmod(c, 2)
        xT = np.ascontiguousarray(x[b].T).astype(np.float16)
        wq = W_attn[:, 384 * g:384 * (g + 1)]
        wk = W_attn[:, C + 384 * g:C + 384 * (g + 1)]
        wv = W_attn[:, 2 * C + 384 * g:2 * C + 384 * (g + 1)]
        wa = np.ascontiguousarray(
            np.concatenate([wq, wk, wv], axis=1)).astype(np.float16)
        wp = np.ascontiguousarray(
            W_proj[384 * g:384 * (g + 1), :]).astype(np.float16)
        m = {"xT": xT, "wa": wa, "wp": wp, "tri": tri}
        if with_bias_qkv:
            m["bqkv"] = np.concatenate(
                [b_attn[384 * g:384 * (g + 1)],
                 b_attn[C + 384 * g:C + 384 * (g + 1)],
                 b_attn[2 * C + 384 * g:2 * C + 384 * (g + 1)]]
            ).reshape(1, -1).astype(np.float16)
        if with_bias_proj:
            m["bp"] = (b_proj / 2.0).reshape(1, -1).astype(np.float16)
        in_maps.append(m)
    return in_maps, with_bias_qkv, with_bias_proj


def unshard_output(results, T):
    out = np.empty((B, T, C), dtype=np.float32)
    splits = SPLITS if T == 2048 else [T // 128]
    for b in range(B):
        for g in range(2):
            r = results[2 * b + g]["out"]
            R = 0    # global row base of split
            acc = 0  # row base within this core's out tensor
            for c in splits:
                rows = 128 * c
                out[b, R + g * rows // 2: R + (g + 1) * rows // 2] = \
                    r[acc: acc + rows // 2]
                R += rows
                acc += rows // 2
    return out


_CACHED = {}


def kernel(x, W_attn, b_attn, W_proj, b_proj):
    x = np.asarray(x, dtype=np.float32)
    W_attn = np.asarray(W_attn, dtype=np.float32)
    b_attn = np.asarray(b_attn, dtype=np.float32)
    W_proj = np.asarray(W_proj, dtype=np.float32)
    b_proj = np.asarray(b_proj, dtype=np.float32)
    T = x.shape[1]
    in_maps, wbq, wbp = shard_inputs(x, W_attn, b_attn, W_proj, b_proj)
    key = (T, wbq, wbp)
    if key not in _CACHED:
        _CACHED[key] = build_program(T, wbq, wbp)
    nc = _CACHED[key]
    res = run_bass_kernel_spmd(nc, in_maps, list(range(N_CORES)))
    return unshard_output(res.results, T)


# revision 22
# speedup vs baseline: 1.0408x; 1.0408x over previous
"""Causal self-attention (B=4, T=2048, C=768, H=12) on 8 trn2 NeuronCores.

Sharding: 4 batches x 2 head-groups = 8 cores. Each core computes the qkv
projection + attention for its 6 heads of one batch element in transposed
layout (q^T,k^T as [hd,T], v as [T,hd] -- zero on-device transposes), a
partial output projection over its 384 y-channels for all T, then pairwise
ReduceScatters (3 splits) sum the two partial projections and hand each core
its rows. All matmul operands are fp16 (PE runs 1 cycle/row, same as f32r,
but 16-bit halves SBUF/DMA traffic and doubles DVE rate); PSUM accumulation
is fp32.

The qkv projection, attention, and output projection are interleaved per
query block so the PE/ACT pipelines fill from ~10us on, and the gpsimd queue
carries only the collectives + final output DMAs so the ReduceScatters
overlap compute.

Host work is limited to slicing/transposing/casting inputs and restacking
outputs.
"""
import numpy as np
from contextlib import ExitStack

import concourse.bass as bass
import concourse.bacc as bacc
import concourse.mybir as mybir
import concourse.tile as tile
from concourse.bass_utils import run_bass_kernel_spmd

B, C, H, HD = 4, 768, 12, 64
N_CORES = 8
LH = H // 2          # local heads per core
F32 = mybir.dt.float32
F16 = mybir.dt.float16
Exp = mybir.ActivationFunctionType.Exp
PAIRS = [[0, 1], [2, 3], [4, 5], [6, 7]]
SPLITS = [8, 4, 4]   # m-chunks (128 rows each) per ReduceScatter


def build_program(T=2048, with_bias_qkv=False, with_bias_proj=False):
    CK = C // 128            # 6 contract chunks of the hidden dim
    QB = min(512, T)         # query block (free dim of S^T)
    NQB = T // QB
    DPB = QB // 128          # 128-wide diagonal strips per query block
    QKCOLS = 2 * LH * HD     # 768 local q+k columns
    VCOLS = LH * HD          # 384 local v columns
    WACOLS = QKCOLS + VCOLS  # 1152
    if T == 2048:
        splits = SPLITS
    else:
        splits = [T // 128]
    assert sum(splits) * 128 == T

    nc = bacc.Bacc("TRN2", target_bir_lowering=False, debug=False,
                   num_devices=N_CORES)
    xT_d = nc.dram_tensor("xT", [C, T], F16, kind="ExternalInput")
    wa_d = nc.dram_tensor("wa", [C, WACOLS], F16, kind="ExternalInput")
    wp_d = nc.dram_tensor("wp", [VCOLS, C], F16, kind="ExternalInput")
    tri_d = nc.dram_tensor("tri", [128, 128], F16, kind="ExternalInput")
    if with_bias_qkv:
        bq_d = nc.dram_tensor("bqkv", [1, WACOLS], F16, kind="ExternalInput")
    if with_bias_proj:
        bp_d = nc.dram_tensor("bp", [1, C], F16, kind="ExternalInput")
    out_d = nc.dram_tensor("out", [T // 2, C], F32, kind="ExternalOutput")

    with tile.TileContext(nc) as tc, ExitStack() as top:
        persist = top.enter_context(tc.tile_pool(name="persist", bufs=1))
        dram = top.enter_context(tc.tile_pool(name="dram", bufs=1, space="DRAM"))

        # persistent tensors
        qkT = [persist.tile([128, T], F16, tag=f"qkT{j}", name=f"qkT{j}") for j in range(CK)]
        kTs = [persist.tile([128, T], F16, tag=f"kTs{j}", name=f"kTs{j}") for j in range(3)]
        yT = [persist.tile([128, T], F16, tag=f"yT{j}", name=f"yT{j}") for j in range(3)]
        v_sb = [persist.tile([128, LH * (HD + 1)], F16, tag=f"v{m}", name=f"v{m}")
                for m in range(T // 128)]
        xt = [persist.tile([128, T], F16, tag=f"xt{i}", name=f"xt{i}") for i in range(CK)]
        wa = [persist.tile([128, WACOLS], F16, tag=f"wa{i}", name=f"wa{i}") for i in range(CK)]
        wp = [persist.tile([128, C], F16, tag=f"wp{j}", name=f"wp{j}") for j in range(3)]
        tri = persist.tile([128, 128], F16, tag="tri")
        nc.sync.dma_start(tri[:], tri_d.ap())
        onescol = persist.tile([128, LH], F16, tag="onescol")
        onescol_f = persist.tile([128, LH], F32, tag="onescol_f")
        nc.vector.memset(onescol_f[:], 1.0)
        nc.vector.tensor_copy(onescol[:], onescol_f[:])
        if with_bias_qkv:
            bq_sb = persist.tile([1, WACOLS], F16, tag="bq")
            nc.sync.dma_start(bq_sb[:], bq_d.ap())
            onesq = persist.tile([1, QB], F16, tag="onesq")
            onesq_f = persist.tile([1, QB], F32, tag="onesq_f")
            nc.vector.memset(onesq_f[:], 1.0)
            nc.vector.tensor_copy(onesq[:], onesq_f[:])
        if with_bias_proj:
            bp_sb = persist.tile([1, C], F16, tag="bp")
            nc.sync.dma_start(bp_sb[:], bp_d.ap())
        if with_bias_qkv or with_bias_proj:
            ones128 = persist.tile([1, 128], F16, tag="ones128")
            ones128_f = persist.tile([1, 128], F32, tag="ones128_f")
            nc.vector.memset(ones128_f[:], 1.0)
            nc.vector.tensor_copy(ones128[:], ones128_f[:])

        # input DMAs, sliced per query block so block 0's chains start early
        for i in range(CK):
            nc.sync.dma_start(wa[i][:], wa_d.ap()[128 * i:128 * (i + 1), :])
        for n in range(NQB):
            for i in range(CK):
                nc.sync.dma_start(xt[i][:, QB * n:QB * (n + 1)],
                                  xT_d.ap()[128 * i:128 * (i + 1),
                                            QB * n:QB * (n + 1)])
        for j in range(3):
            nc.sync.dma_start(wp[j][:], wp_d.ap()[128 * j:128 * (j + 1), :])

        # collectives state
        partials = [dram.tile([128 * c, C], F16, name=f"partial{i}")
                    for i, c in enumerate(splits)]
        rs_outs = [dram.tile([64 * c, C], F16, name=f"rs{i}")
                   for i, c in enumerate(splits)]
        split_of_m = []
        for i, c in enumerate(splits):
            split_of_m += [i] * c
        split_base = [0] * len(splits)
        for i in range(1, len(splits)):
            split_base[i] = split_base[i - 1] + splits[i - 1]
        rs_done = [0] * len(splits)
        out_row = [0] * (len(splits) + 1)
        for i, c in enumerate(splits):
            out_row[i + 1] = out_row[i] + 64 * c

        with tc.tile_pool(name="phB", bufs=6) as phB, \
             tc.tile_pool(name="phBs", bufs=4) as phBs, \
             tc.tile_pool(name="stg", bufs=4) as stg, \
             tc.tile_pool(name="psS", bufs=3, space="PSUM") as psS, \
             tc.tile_pool(name="psY", bufs=2, space="PSUM") as psY:

            # ACT spline-table preload: a throwaway exp overlapping the input
            # DMAs instead of stalling the first real softmax tile
            dummy = phBs.tile([1, 16], F32, tag="dummy", bufs=1)
            nc.vector.memset(dummy[:], 0.0)
            dume = phBs.tile([1, 16], F32, tag="dume", bufs=1)
            nc.scalar.activation(dume[:], dummy[:], Exp, scale=0.125)

            # warm the PE clock-gate while input DMAs stream
            for w in range(8):
                wps = psS.tile([128, 2 * QB], F32, tag="sps", bufs=3,
                               name=f"warm{w}")
                nc.tensor.matmul(wps[:, 0:128], tri[:], tri[:],
                                 start=True, stop=True)

            def v_chain(m):
                vps = psY.tile([128, QB], F32, tag="yps", name=f"vps{m}")
                for i in range(CK):
                    nc.tensor.matmul(
                        vps[:, 0:VCOLS], xt[i][:, 128 * m:128 * (m + 1)],
                        wa[i][:, QKCOLS:WACOLS],
                        start=(i == 0),
                        stop=(i == CK - 1 and not with_bias_qkv))
                if with_bias_qkv:
                    nc.tensor.matmul(vps[:, 0:VCOLS], ones128[:],
                                     bq_sb[:, QKCOLS:WACOLS],
                                     start=False, stop=True)
                nc.vector.tensor_copy(
                    v_sb[m][:].rearrange("p (h c) -> p h c", c=HD + 1)[:, :, 0:HD],
                    vps[:, 0:VCOLS].rearrange("p (h c) -> p h c", c=HD))
                nc.vector.tensor_copy(
                    v_sb[m][:].rearrange("p (h c) -> p h c", c=HD + 1)[:, :, HD:HD + 1],
                    onescol[:].rearrange("p (h c) -> p h c", c=1))

            for n in range(NQB):
                # ---- phase A for this block: q_j + k_j chains together so
                # head pair (2j, 2j+1) unblocks as soon as its pair is done
                for jp in range(3):
                    qpw = psS.tile([128, 2 * QB], F32, tag="sps", bufs=3,
                                   name=f"qp{n}_{jp}")
                    for half, j in ((0, jp), (1, 3 + jp)):
                        for i in range(CK):
                            nc.tensor.matmul(
                                qpw[:, QB * half:QB * (half + 1)],
                                wa[i][:, 128 * j:128 * (j + 1)],
                                xt[i][:, QB * n:QB * (n + 1)],
                                start=(i == 0),
                                stop=(i == CK - 1 and not with_bias_qkv))
                        if with_bias_qkv:
                            nc.tensor.matmul(
                                qpw[:, QB * half:QB * (half + 1)],
                                bq_sb[:, 128 * j:128 * (j + 1)], onesq[:],
                                start=False, stop=True)
                        nc.vector.tensor_copy(
                            qkT[j][:, QB * n:QB * (n + 1)],
                            qpw[:, QB * half:QB * (half + 1)])
                    # half-swapped copy of k^T so both PE row groups can host
                    # any head
                    cs = QB * n
                    nc.sync.dma_start(kTs[jp][64:128, cs:cs + QB],
                                      qkT[3 + jp][0:64, cs:cs + QB])
                    nc.sync.dma_start(kTs[jp][0:64, cs:cs + QB],
                                      qkT[3 + jp][64:128, cs:cs + QB])
                for m in range(DPB * n, DPB * (n + 1)):
                    v_chain(m)

                # ---- attention for this block
                nkc = DPB * (n + 1)
                for h in range(LH):
                    jq, rq = h // 2, 64 * (h % 2)
                    klo = qkT[3 + jq] if h % 2 == 0 else kTs[jq]
                    khi = kTs[jq] if h % 2 == 0 else qkT[3 + jq]
                    yps = psY.tile([128, QB], F32, tag="yps",
                                   name=f"yps{n}_{h}")
                    # stage this (head, block)'s q at the opposite base
                    ob = 64 - rq
                    qst = phB.tile([128, QB], F16, tag="qst", bufs=2)
                    nc.sync.dma_start(
                        qst[ob:ob + 64, :],
                        qkT[jq][rq:rq + 64, QB * n:QB * (n + 1)])
                    for kc0 in range(0, nkc, 2):
                        pair = [kc0] if kc0 + 1 >= nkc else [kc0, kc0 + 1]
                        # both S^T tiles of the pair land in one 2-bank psum
                        # tile; adjacent matmuls in distinct PE row groups run
                        # concurrently (K=64 row tiling)
                        spw = psS.tile([128, 2 * QB], F32, tag="sps", bufs=3,
                                       name=f"sp{n}_{h}_{kc0}")
                        ptw = phB.tile([128, 2 * QB], F16, tag="pt", bufs=4)
                        offs = []
                        for pi, kc in enumerate(pair):
                            d = kc - DPB * n
                            c0 = 128 * d if d > 0 else 0
                            off = pi * QB
                            offs.append((kc, d, c0, off))
                            kt, rb = (klo, 0) if kc % 2 == 0 else (khi, 64)
                            if rb == rq:
                                qt_ap = qkT[jq][rq:rq + 64,
                                                QB * n + c0:QB * (n + 1)]
                            else:
                                qt_ap = qst[ob:ob + 64, c0:QB]
                            nc.tensor.matmul(
                                spw[:, off + c0:off + QB],
                                kt[rb:rb + 64, 128 * kc:128 * (kc + 1)],
                                qt_ap, start=True, stop=True)
                        # one wide exp when the garbage prefix is small
                        if len(pair) == 2 and sum(c for _, _, c, _ in offs) <= 256:
                            lo = offs[0][2]
                            nc.scalar.activation(ptw[:, lo:], spw[:, lo:],
                                                 Exp, scale=0.125)
                        else:
                            for kc, d, c0, off in offs:
                                nc.scalar.activation(
                                    ptw[:, off + c0:off + QB],
                                    spw[:, off + c0:off + QB],
                                    Exp, scale=0.125)
                        for kc, d, c0, off in offs:
                            if d >= 0:
                                nc.vector.tensor_tensor(
                                    ptw[:, off + c0:off + c0 + 128],
                                    ptw[:, off + c0:off + c0 + 128],
                                    tri[:], mybir.AluOpType.mult)
                        for kc, d, c0, off in offs:
                            nc.tensor.matmul(
                                yps[0:HD + 1, c0:QB],
                                v_sb[kc][:, (HD + 1) * h:(HD + 1) * (h + 1)],
                                ptw[:, off + c0:off + QB],
                                start=(kc == 0), stop=(kc == nkc - 1))
                    # normalize: yT[.] = yps[0:64] / yps[64].  One staging
                    # copy frees the PSUM tile; reciprocal on DVE; partition
                    # broadcast via a DRAM round-trip DMA (engines cannot
                    # read stride-0 partitions, DMA from DRAM can; gpsimd
                    # stays collective-only)
                    ystg = phBs.tile([HD + 1, QB], F32, tag="ystg", bufs=4,
                                     name=f"ystg{n}_{h}")
                    nc.vector.tensor_copy(ystg[:], yps[0:HD + 1, :])
                    # the custom-DVE reciprocal needs a partition-0 input AP
                    s_sb = phBs.tile([1, QB], F32, tag="s_sb", bufs=4,
                                     name=f"s_sb{n}_{h}")
                    nc.vector.tensor_copy(s_sb[:], yps[HD:HD + 1, :])
                    rec = phBs.tile([1, QB], F32, tag="rec", bufs=4,
                                    name=f"rec{n}_{h}")
                    nc.vector.reciprocal_approx_fast(rec[:], s_sb[:])
                    recd = dram.tile([1, QB], F32, tag="recd", bufs=4,
                                     name=f"recd{n}_{h}")
                    nc.sync.dma_start(recd[:], rec[:])
                    recb = phBs.tile([HD, QB], F32, tag="recb", bufs=4,
                                     name=f"recb{n}_{h}")
                    nc.sync.dma_start(recb[:], recd[:].to_broadcast((HD, QB)))
                    nc.vector.tensor_tensor(
                        yT[jq][rq:rq + 64, QB * n:QB * (n + 1)],
                        ystg[0:HD, :], recb[:], mybir.AluOpType.mult)

                # ---- output projection for this block's row chunks
                for m in range(DPB * n, DPB * (n + 1)):
                    ost = stg.tile([128, C], F16, tag="ost", bufs=4,
                                   name=f"ost{m}")
                    for c0, c1 in ((0, 512), (512, C)):
                        pps = psY.tile([128, QB], F32, tag="yps",
                                       name=f"pp{m}_{c0}")
                        for j in range(3):
                            nc.tensor.matmul(
                                pps[:, 0:c1 - c0],
                                yT[j][:, 128 * m:128 * (m + 1)],
                                wp[j][:, c0:c1], start=(j == 0),
                                stop=(j == 2 and not with_bias_proj))
                        if with_bias_proj:
                            nc.tensor.matmul(pps[:, 0:c1 - c0], ones128[:],
                                             bp_sb[:, c0:c1],
                                             start=False, stop=True)
                        nc.vector.tensor_copy(ost[:, c0:c1],
                                              pps[:, 0:c1 - c0])
                    sp = split_of_m[m]
                    mh = m - split_base[sp]
                    nc.sync.dma_start(
                        partials[sp][128 * mh:128 * (mh + 1), :], ost[:])
                    rs_done[sp] += 1
                    if rs_done[sp] == splits[sp]:
                        nc.gpsimd.collective_compute(
                            "ReduceScatter", mybir.AluOpType.add,
                            replica_groups=PAIRS,
                            ins=[partials[sp].opt()],
                            outs=[rs_outs[sp].opt()])
                        nc.gpsimd.dma_start(
                            out=out_d.ap()[out_row[sp]:out_row[sp + 1], :],
                            in_=rs_outs[sp][:, :])
    nc.compile()
    return nc


def shard_inputs(x, W_attn, b_attn, W_proj, b_proj):
    """Per-core input maps. Core c = 2*b + g handles batch b, head-group g."""
    T = x.shape[1]
    tri = np.tril(np.ones((128, 128), dtype=np.float32)).T.copy()
    # tri[k_row, q_col] = 1 where k <= q  (lower-tri in (q,k) = upper in (k,q))
    tri = tri.astype(np.float16)
    with_bias_qkv = bool(np.any(b_attn))
    with_bias_proj = bool(np.any(b_proj))
    in_maps = []
    for c in range(N_CORES):
        b, g = divmod(c, 2)
        xT = np.ascontiguousarray(x[b].T).astype(np.float16)
        wq = W_attn[:, 384 * g:384 * (g + 1)]
        wk = W_attn[:, C + 384 * g:C + 384 * (g + 1)]
        wv = W_attn[:, 2 * C + 384 * g:2 * C + 384 * (g + 1)]
        wa = np.ascontiguousarray(
            np.concatenate([wq, wk, wv], axis=1)).astype(np.float16)
        wp = np.ascontiguousarray(
            W_proj[384 * g:384 * (g + 1), :]).astype(np.float16)
        m = {"xT": xT, "wa": wa, "wp": wp, "tri": tri}
        if with_bias_qkv:
            m["bqkv"] = np.concatenate(
                [b_attn[384 * g:384 * (g + 1)],
                 b_attn[C + 384 * g:C + 384 * (g + 1)],
                 b_attn[2 * C + 384 * g:2 * C + 384 * (g + 1)]]
            ).reshape(1, -1).astype(np.float16)
        if with_bias_proj:
            m["bp"] = (b_proj / 2.0).reshape(1, -1).astype(np.float16)
        in_maps.append(m)
    return in_maps, with_bias_qkv, with_bias_proj


def unshard_output(results, T):
    out = np.empty((B, T, C), dtype=np.float32)
    splits = SPLITS if T == 2048 else [T // 128]
    for b in range(B):
        for g in range(2):
            r = results[2 * b + g]["out"]
            R = 0    # global row base of split
            acc = 0  # row base within this core's out tensor
            for c in splits:
                rows = 128 * c
                out[b, R + g * rows // 2: R + (g + 1) * rows // 2] = \
                    r[acc: acc + rows // 2]
                R += rows
                acc += rows // 2
    return out


_CACHED = {}


def kernel(x, W_attn, b_attn, W_proj, b_proj):
    x = np.asarray(x, dtype=np.float32)
    W_attn = np.asarray(W_attn, dtype=np.float32)
    b_attn = np.asarray(b_attn, dtype=np.float32)
    W_proj = np.asarray(W_proj, dtype=np.float32)
    b_proj = np.asarray(b_proj, dtype=np.float32)
    T = x.shape[1]
    in_maps, wbq, wbp = shard_inputs(x, W_attn, b_attn, W_proj, b_proj)
    key = (T, wbq, wbp)
    if key not in _CACHED:
        _CACHED[key] = build_program(T, wbq, wbp)
    nc = _CACHED[key]
    res = run_bass_kernel_spmd(nc, in_maps, list(range(N_CORES)))
    return unshard_output(res.results, T)
